# revision 1
# baseline (speedup 1.0000x reference)
"""Multi-head causal attention (B=4, S=2048, D=1024, 16 heads) on 8 TRN2 cores.

Sharding: core c -> (batch b = c//2, head-group g = c%2). Each core computes
8 heads of one batch element end-to-end (QKV proj, causal softmax attention,
out-proj rows for its head slice). Host sums the two head-group partials per
batch and adds the output bias.

Per-core pipeline (all matmuls contraction-on-partitions, bf16 in / f32 psum):
  QT/KT[dtile] = (x @ w)^T   [128p = 2 heads x 64, S]
  Vones[kb]    = [V | 1]     [128p = k, h, 65]
  attention per (512-wide q-chunk, head-pair); the pair's score matmuls are
  packed into PE row groups via tile_position (concurrent on real HW); two
  k-blocks share one [128,1024] score psum so each exp covers ~1024 cols;
  PV matmuls trail two iterations behind so PE never waits on the exp:
    ST[k,q] = KT.T @ QT; PT = exp(ST/8) bf16; tri-mask on diagonal 128 cols
    ctx[65, 512] += [V|1].T @ PT   (row 64 = softmax denominators)
    cxt = ctx[0:64] * gpsimd-broadcast(1/ctx[64])
  out[seq128, 512] = cxt.T @ ow, streamed to DRAM per q-chunk; the final
  q-chunk's out-proj borrows the freed score psum banks.
"""

import numpy as np
import ml_dtypes

B, S, D = 4, 2048, 1024
H_TOT = 16
HD = 64
NCORES = 8
GH = 8          # heads per core
GD = GH * HD    # 512: dout slice per core
NKB = S // 128  # 16 k-blocks
NQC = S // 512  # 4 q-chunks
BF16 = ml_dtypes.bfloat16

PACK_HEADS = True   # pack 2 heads' score matmuls into PE row groups

_cache = {}


def _build_body(tc, nc, mybir, xT, wq, wk, wv, ow, outp):
    from concourse.masks import make_upper_triangular
    import contextlib

    dt = mybir.dt
    F = mybir.ActivationFunctionType

    pools = contextlib.ExitStack()
    tc_pool = lambda **kw: pools.enter_context(tc.tile_pool(**kw))

    singles = tc_pool(name="singles", bufs=1)
    pt_pool = tc_pool(name="pt", bufs=8)
    small = tc_pool(name="small", bufs=4)
    rb_pool = tc_pool(name="rb", bufs=6)
    ost_pool = tc_pool(name="ost", bufs=5)
    psum_st = tc_pool(name="psum_st", bufs=2, space="PSUM")
    psum_ctx = tc_pool(name="psum_ctx", bufs=3, space="PSUM")
    psum_mm = tc_pool(name="psum_mm", bufs=1, space="PSUM")

    # ---- persistent SBUF tensors (split per producer/consumer region so the
    # dependency tracker never over-serializes) ----
    xT_sb = [singles.tile([128, S], dt.bfloat16, name=f"xt{t}")
             for t in range(8)]
    wq_sb = [singles.tile([128, GD], dt.bfloat16, name=f"wq{t}")
             for t in range(8)]
    wk_sb = [singles.tile([128, GD], dt.bfloat16, name=f"wk{t}")
             for t in range(8)]
    wv_sb = [singles.tile([128, GD], dt.bfloat16, name=f"wv{t}")
             for t in range(8)]
    ow_sb = [singles.tile([128, D], dt.bfloat16, name=f"ow{t}")
             for t in range(4)]
    qt_sb = [singles.tile([128, S], dt.bfloat16, name=f"qt{t}")
             for t in range(4)]                              # 2 heads / dtile
    kt_sb = [singles.tile([128, S], dt.bfloat16, name=f"kt{t}")
             for t in range(4)]
    vo_sb = [singles.tile([128, GH, 65], dt.bfloat16, name=f"vo{t}")
             for t in range(NKB)]                            # [V_h | ones]
    cxt_sb = [singles.tile([128, S], dt.bfloat16, name=f"cxt{t}")
              for t in range(4)]                             # ctx^T normalized
    tri = singles.tile([128, 128], dt.bfloat16)              # keep k<=q

    make_upper_triangular(nc, tri, val=1.0, diag=True)
    for t in range(NKB):
        nc.vector.memset(vo_sb[t][:, :, 64:65], 1.0)

    # ---- input DMAs (split per 128-row tile; first matmuls need wq t0 + xT t0) ----
    xT_r = xT.ap().rearrange("(t p) s -> p t s", p=128)
    wq_r = wq.ap().rearrange("(t p) n -> p t n", p=128)
    wk_r = wk.ap().rearrange("(t p) n -> p t n", p=128)
    wv_r = wv.ap().rearrange("(t p) n -> p t n", p=128)
    ow_r = ow.ap().rearrange("(t p) n -> p t n", p=128)
    # SP queue: even xT tiles then wk; ACT queue: wq (small) then odd xT;
    # gpsimd SWDGE: wv/ow (needed late). Two HWDGE queues halve the
    # serial input-load latency the first projections wait on.
    DIN_ORDER = list(range(8))
    for t in range(8):
        nc.sync.dma_start(out=xT_sb[t], in_=xT_r[:, t, :])
        nc.sync.dma_start(out=wq_sb[t], in_=wq_r[:, t, :])
        if t % 2 == 1:
            nc.sync.dma_start(out=wk_sb[t // 2], in_=wk_r[:, t // 2, :])
    for t in range(4, 8):
        nc.sync.dma_start(out=wk_sb[t], in_=wk_r[:, t, :])
    for t in range(8):
        nc.sync.dma_start(out=wv_sb[t], in_=wv_r[:, t, :])
    for t in range(4):
        nc.sync.dma_start(out=ow_sb[t], in_=ow_r[:, t, :])

    def emit_proj_dtile(w_sb, t_sb, dtile):
        # din-outer: tolerate in-flight xT DMAs; 2 stp slots = 4 psum halves
        pst = [psum_st.tile([128, 1024], dt.float32, name="stp")
               for _ in range(2)]
        pss = [pst[0][:, 0:512], pst[0][:, 512:1024],
               pst[1][:, 0:512], pst[1][:, 512:1024]]
        for i, din in enumerate(DIN_ORDER):
            for c in range(4):
                nc.tensor.matmul(
                    pss[c],
                    lhsT=w_sb[din][:, dtile * 128:(dtile + 1) * 128],
                    rhs=xT_sb[din][:, c * 512:(c + 1) * 512],
                    start=(i == 0),
                    stop=(i == 7),
                )
        for c in range(4):
            nc.vector.tensor_copy(
                out=t_sb[dtile][:, c * 512:(c + 1) * 512], in_=pss[c])

    def emit_v(st):
        ps = psum_mm.tile([128, 512], dt.float32, name="mmps")
        for din in range(8):
            nc.tensor.matmul(
                ps,
                lhsT=xT_sb[din][:, st * 128:(st + 1) * 128],
                rhs=wv_sb[din],
                start=(din == 0),
                stop=(din == 7),
            )
        nc.vector.tensor_copy(
            out=vo_sb[st][:, :, 0:64],
            in_=ps.rearrange("p (h d) -> p h d", h=GH),
        )

    def emit_norm(ctx_tile, h, q0):
        """cxt[h rows, q0:q0+512] = ctx[0:64] * broadcast(1/ctx[64])."""
        recip = small.tile([1, 512], dt.float32, name="recip")
        nc.vector.reciprocal(out=recip, in_=ctx_tile[64:65, :])
        rb = rb_pool.tile([64, 512], dt.float32, name="rb")
        nc.gpsimd.partition_broadcast(rb, recip)
        nc.vector.tensor_mul(
            cxt_sb[h // 2][(h % 2) * 64:(h % 2) * 64 + 64, q0:q0 + 512],
            ctx_tile[0:64, :],
            rb,
        )

    def emit_attn_pair(qc, hp):
        """512-wide q chunk qc for heads h0=2*hp (array rows 0:64) and
        h1=2*hp+1 (rows 64:128); scores packed into PE row groups.

        Two k-blocks share one [128,1024] ST psum tile so each exp covers up
        to 1024 columns. PV matmuls trail by one pair-iteration so the PE
        never sits directly behind the exp on the ACT engine."""
        nkb = 4 * qc + 4
        q0 = 512 * qc
        ctxs = [psum_ctx.tile([65, 512], dt.float32, name="ctx")
                for _ in range(2)]
        pend2 = []

        def emit_pv(kbs, offs, ns, pts):
            for half in range(2):
                for (kb, off, n) in zip(kbs, offs, ns):
                    nc.tensor.matmul(
                        ctxs[half][:, 512 - n:512],
                        lhsT=vo_sb[kb][:, 2 * hp + half, :],
                        rhs=pts[half][:, off:off + n],
                        start=(kb == 0),
                        stop=(kb == nkb - 1),
                    )

        for kb0 in range(0, nkb, 2):
            kbs = [kb for kb in (kb0, kb0 + 1) if kb < nkb]
            ns = [512 - max(0, kb * 128 - q0) for kb in kbs]
            offs = [0] + [ns[0]] * (len(kbs) - 1)
            pts = []
            for half in range(2):
                p0 = half * 64
                stp = psum_st.tile([128, 1024], dt.float32, name="stp")
                for kb, off, n in zip(kbs, offs, ns):
                    nc.tensor.matmul(
                        stp[:, off:off + n],
                        lhsT=kt_sb[hp][p0:p0 + 64, kb * 128:(kb + 1) * 128],
                        rhs=qt_sb[hp][p0:p0 + 64, q0 + 512 - n:q0 + 512],
                        start=True,
                        stop=True,
                        tile_position=(p0, 0) if PACK_HEADS else None,
                    )
                ntot = offs[-1] + ns[-1]
                pt = pt_pool.tile([128, 1024], dt.bfloat16, name="pt")
                nc.scalar.activation(
                    out=pt[:, :ntot], in_=stp[:, :ntot], func=F.Exp,
                    scale=0.125)
                for kb, off in zip(kbs, offs):
                    if kb >= 4 * qc:  # diagonal: mask first 128 cols
                        nc.vector.tensor_mul(
                            pt[:, off:off + 128], pt[:, off:off + 128], tri)
                pts.append(pt)
            pend2.append((kbs, offs, ns, pts))
            if len(pend2) > 2:
                emit_pv(*pend2.pop(0))
        for p in pend2:
            emit_pv(*p)
        for half in range(2):
            emit_norm(ctxs[half], 2 * hp + half, q0)

    def emit_p4(sq_lo, sq_hi, final=False):
        for sq in range(sq_lo, sq_hi):
            for oc in range(2):
                if final:  # attention done: rotate over ALL freed banks
                    k = (sq * 2 + oc) % 3
                    if k == 0:
                        ps = psum_st.tile([128, 1024], dt.float32,
                                          name="stp")[:, 0:512]
                    elif k == 1:
                        ps = psum_ctx.tile([128, 512], dt.float32, name="ctx")
                    else:
                        ps = psum_mm.tile([128, 512], dt.float32, name="mmps")
                else:
                    ps = psum_mm.tile([128, 512], dt.float32, name="mmps")
                for dvt in range(4):
                    nc.tensor.matmul(
                        ps,
                        lhsT=cxt_sb[dvt][:, sq * 128:(sq + 1) * 128],
                        rhs=ow_sb[dvt][:, oc * 512:(oc + 1) * 512],
                        start=(dvt == 0),
                        stop=(dvt == 3),
                    )
                ost = ost_pool.tile([128, 512], dt.float32, name="ost")
                nc.vector.tensor_copy(out=ost, in_=ps)
                nc.sync.dma_start(
                    out=outp.ap()[sq * 128:(sq + 1) * 128,
                                  oc * 512:(oc + 1) * 512],
                    in_=ost,
                )

    # ---- emission schedule: pipeline projections with qc=0 attention ----
    v_ranges = [range(0, 4), range(4, 8), range(8, 12), range(12, 16)]
    for dtile in range(4):
        emit_proj_dtile(wq_sb, qt_sb, dtile)
        emit_proj_dtile(wk_sb, kt_sb, dtile)
        for st in v_ranges[dtile]:
            emit_v(st)
        emit_attn_pair(0, dtile)
    for qc in range(1, NQC):
        for hp in range(4):
            emit_attn_pair(qc, hp)
            if hp == 0:
                emit_p4(4 * (qc - 1), 4 * qc)
    emit_p4(12, 16, final=True)

    return pools


def _build_nc():
    import concourse.tile as tile
    from concourse import bacc, mybir

    dt = mybir.dt
    nc = bacc.Bacc("TRN2", target_bir_lowering=False, debug=False,
                   num_devices=NCORES)
    xT = nc.dram_tensor("xt", [D, S], dt.bfloat16, kind="ExternalInput")
    wq = nc.dram_tensor("wq", [D, GD], dt.bfloat16, kind="ExternalInput")
    wk = nc.dram_tensor("wk", [D, GD], dt.bfloat16, kind="ExternalInput")
    wv = nc.dram_tensor("wv", [D, GD], dt.bfloat16, kind="ExternalInput")
    ow = nc.dram_tensor("ow", [GD, D], dt.bfloat16, kind="ExternalInput")
    outp = nc.dram_tensor("outp", [S, D], dt.float32, kind="ExternalOutput")

    with tile.TileContext(nc) as tc:
        pools = _build_body(tc, nc, mybir, xT, wq, wk, wv, ow, outp)
        pools.close()
    nc.compile()
    return nc


LAST_RESULTS = None


def kernel(batch, w_query, w_key, w_value, out_w, out_b):
    global LAST_RESULTS
    import os
    from concourse import bass_utils

    try:  # BASS_TRACE needs the axon NTFF hook; without it the run crashes
        from antenv.axon_hooks import get_axon_ntff_profile_hook  # noqa: F401
    except ImportError:
        os.environ.setdefault("BASS_NEVER_TRACE", "1")

    batch = np.asarray(batch, dtype=np.float32)
    w_query = np.asarray(w_query, dtype=np.float32)
    w_key = np.asarray(w_key, dtype=np.float32)
    w_value = np.asarray(w_value, dtype=np.float32)
    out_w = np.asarray(out_w, dtype=np.float32)
    out_b = np.asarray(out_b, dtype=np.float32)

    if "nc" not in _cache:
        _cache["nc"] = _build_nc()
    nc = _cache["nc"]

    xts = [np.ascontiguousarray(batch[b].T).astype(BF16) for b in range(B)]
    slc = [slice(g * GD, (g + 1) * GD) for g in range(2)]
    wqs = [np.ascontiguousarray(w_query[:, s]).astype(BF16) for s in slc]
    wks = [np.ascontiguousarray(w_key[:, s]).astype(BF16) for s in slc]
    wvs = [np.ascontiguousarray(w_value[:, s]).astype(BF16) for s in slc]
    ows = [np.ascontiguousarray(out_w[s, :]).astype(BF16) for s in slc]
    in_maps = []
    for c in range(NCORES):
        b, g = divmod(c, 2)
        in_maps.append({
            "xt": xts[b], "wq": wqs[g], "wk": wks[g],
            "wv": wvs[g], "ow": ows[g],
        })

    res = bass_utils.run_bass_kernel_spmd(
        nc, in_maps, core_ids=list(range(NCORES)),
    )
    LAST_RESULTS = res

    out = np.empty((B, S, D), np.float32)
    for b in range(B):
        out[b] = res.results[2 * b]["outp"] + res.results[2 * b + 1]["outp"] \
            + out_b[None, :]
    return out

